# revision 19
# baseline (speedup 1.0000x reference)
"""Trainium2 Bass kernel for nn_DynamicUpsamplePAC (v2: pair-drain scheme).

Math (per batch item, data-parallel over B=8 -> 8 cores):
  1. x_d = bilinear downsample 160->128 (torch align_corners=False), fp16,
     held zero-padded as xd[c, 130, 130] (scale-deferred; undone in kern).
  2. kern[tau, t, w'] = exp(-0.5||g_nb - g_c||^2) at the 25 used
     (class, tap) combos; tau = 5i+j lex order; kern[12] == 1.
  3. Parity decomposition: out pixel (2t+al, 2w'+be) sums taps (i,j) with
     i%2==al, j%2==be reading x_d at (t+dt(i), w'+ds(j)),
     dt = [-1,0,0,1,1][i], ds = [-1,0,0,1,1][j].

Phase C v2 (the change vs v1):
  - Columns processed in PAIRS (u=2k,2k+1) with two single-buffered PSUM
    half-tiles P0/P1 = [pair(2), tau(25), o-half(32)] that ping-pong:
    while ACT drains P0, PE matmuls fill P1.  Drains write the ring
    ybuf[t?au, o, slot] with an innermost slot-PAIR (stride-1 run of 2
    fp16 = 4B), which measures ~1.26 ns/elem vs 4.6 ns/elem for the v1
    single-slot (stride-26) scatter that made ScalarE 88% busy.
  - WB=16 w'-blocks (8 blocks): DVE mults (2x fp16, w-innermost, kern
    o-broadcast middle), tap-sum adds on GpSimd, final i-add + bias on
    DVE writing fp32 directly, one output DMA per parity class with a
    strided DRAM view (be-interleave done by the DMA, not GpSimd).
"""

import numpy as np

B, C_IN, C_OUT, K = 8, 128, 64, 5
H_IN = W_IN = 160
T = 256
N_CORES = 8
R = 24          # ybuf ring slots (even; >= WB + 8)
WB = 16         # w'-block width
NBLK = 128 // WB

_FRACS = [0.125, 0.375, 0.625, 0.875]
_DT = [-1, 0, 0, 1, 1]

# matmul chunks per pair-column position: (tau0, ntau, dt), each chunk
# within one 512-fp32 PSUM bank given the half-tile base (c*800).
_CH = [
    [(0, 5, -1), (5, 10, 0), (15, 1, 1), (16, 9, 1)],             # c=0
    [(0, 5, -1), (5, 2, 0), (7, 8, 0), (15, 8, 1), (23, 2, 1)],   # c=1
]

DVE_DRAIN_MOD = 999   # pairs k with k%MOD==PHASE get h=1 drained on DVE
DVE_DRAIN_PHASE = 3


def _build_program(reps=1):
    import concourse.bass as bass  # noqa: F401
    import concourse.mybir as mybir
    import concourse.tile as tile
    from concourse import bacc

    f32 = mybir.dt.float32
    f16 = mybir.dt.float16
    i32 = mybir.dt.int32
    Alu = mybir.AluOpType
    Act = mybir.ActivationFunctionType

    nc = bacc.Bacc("TRN2", target_bir_lowering=False, debug=False,
                   enable_asserts=False)

    x_d = nc.dram_tensor("x", [C_IN, H_IN, W_IN], f32, kind="ExternalInput").ap()
    g_d = nc.dram_tensor("guide", [3, T, T], f32, kind="ExternalInput").ap()
    w_d = nc.dram_tensor("weight", [C_IN, C_OUT, K, K], f32, kind="ExternalInput").ap()
    b_d = nc.dram_tensor("bias", [C_OUT], f32, kind="ExternalInput").ap()
    o_d = nc.dram_tensor("out", [C_OUT, T, T], f32, kind="ExternalOutput").ap()

    with tile.TileContext(nc) as tc:
      for _rep in range(reps):
        with tc.tile_pool(name="pers", bufs=1) as pers, \
             nc.allow_low_precision("fp16 pipeline, fp32 dyn range ok"):
            xd = pers.tile([128, 130 * 130], f16, tag="xd")
            xd3 = xd[:].rearrange("p (v s) -> p v s", s=130)
            kern = pers.tile([128, 25 * 128], f16, tag="kern")
            wb = pers.tile([128, 25 * 64], f16, tag="wb")
            wb3 = wb[:].rearrange("p (t o) -> p t o", o=64)
            bias16 = pers.tile([128, 64], f16, tag="bias16")
            # ybuf is allocated after phases A/B (lifetime-disjoint with
            # their big scratch) -- see phase C below.

            # ---- weights: contig DMA + one ACT reorder/cast ----
            with tc.tile_pool(name="wld", bufs=1) as pw:
                wst = pw.tile([128, 1600], f32, tag="wst")
                nc.sync.dma_start(out=wst[:],
                                  in_=w_d.rearrange("c o i j -> c (o i j)"))
                wst4 = wst[:].rearrange("p (o i j) -> p o i j", i=K, j=K)
                wb4 = wb[:].rearrange("p (i j o) -> p i j o", i=K, j=K)
                nc.scalar.copy(out=wb4, in_=wst4.rearrange("p o i j -> p i j o"))

                bias_bc = pw.tile([128, 64], f32, tag="bias_bc")
                b_src = b_d.rearrange("(o a) -> o a", a=1) \
                           .broadcast_to([64, 128]).rearrange("o r -> r o")
                nc.sync.dma_start(out=bias_bc[:], in_=b_src)
                nc.vector.tensor_copy(bias16[:], bias_bc[:])

            # ---- phase A: bilinear downsample -> xd fp16, scale-deferred --
            # xd[c, row, phys] = true/(f(r)*g(rw)); cols rw-major:
            # phys(u) = 1 + 32*rw + m for data col u-1 = 4m+rw; pads at 0/129.
            # The deferred 1/(f*g) is folded into kern in phase B.
            nc.gpsimd.memset(xd3[:, 0, :], 0.0)
            nc.gpsimd.memset(xd3[:, 129, :], 0.0)
            nc.gpsimd.memset(xd3[:, :, 0], 0.0)
            nc.gpsimd.memset(xd3[:, :, 129], 0.0)
            with tc.tile_pool(name="phAw", bufs=1) as pAw, \
                 tc.tile_pool(name="phA", bufs=2) as pA:
                xw = pAw.tile([128, 160 * 128], f16, tag="xw")
                xw4 = xw[:].rearrange("p (r rw m) -> p r rw m", rw=4, m=32)
                for ch in range(5):
                    xh = pA.tile([128, 32 * 160], f32, tag="xh")
                    xh3 = xh[:].rearrange("p (r w) -> p r w", w=160)
                    nc.sync.dma_start(out=xh3,
                                      in_=x_d[:, 32 * ch:32 * ch + 32, :])
                    for rw in range(4):
                        g_rw = _FRACS[rw]
                        c2 = (1.0 - g_rw) / g_rw
                        aw = xh3[:, :, rw:rw + 156:5]
                        bw = xh3[:, :, rw + 1:rw + 157:5]
                        nc.vector.scalar_tensor_tensor(
                            out=xw4[:, 32 * ch:32 * ch + 32, rw, :],
                            in0=aw, scalar=c2, in1=bw,
                            op0=Alu.mult, op1=Alu.add)
                xw3 = xw[:].rearrange("p (r w) -> p r w", w=128)
                for r in range(4):
                    c_r = (1.0 - _FRACS[r]) / _FRACS[r]
                    a_v = xw3[:, r:r + 156:5, :]
                    b_v = xw3[:, r + 1:r + 157:5, :]
                    dst = xd3[:, 1 + r:1 + r + 125:4, 1:129]
                    nc.vector.scalar_tensor_tensor(
                        out=dst, in0=a_v, scalar=c_r, in1=b_v,
                        op0=Alu.mult, op1=Alu.add)

            # ---- phase B: pac kernel -> kern [t, tau, 128] fp16 ----
            # kern is pre-scaled by f((t+dt)%4)*g((wp+ds)%4) to undo the
            # deferred bilinear scales living in xd.
            with tc.tile_pool(name="phB", bufs=1) as pB:
                grow = {}
                for dsv in (-1, 0, 1):
                    gtile = pB.tile([128, 128], f16, tag=f"grow{dsv}")
                    g3 = gtile[:].rearrange("p (m k) -> p m k", k=4)
                    for k in range(4):
                        nc.gpsimd.memset(g3[:, :, k],
                                         float(_FRACS[(k + dsv) % 4]))
                    grow[dsv] = gtile
                fcol = {}
                fi = pB.tile([128, 1], i32, tag="fi")
                ff = pB.tile([128, 1], f32, tag="ff")
                for dtv in (-1, 0, 1):
                    nc.gpsimd.iota(fi[:], pattern=[[0, 1]], base=dtv + 4,
                                   channel_multiplier=1)
                    nc.vector.tensor_scalar(out=fi[:], in0=fi[:], scalar1=3,
                                            scalar2=None,
                                            op0=Alu.bitwise_and)
                    nc.vector.tensor_copy(ff[:], fi[:])
                    ftile = pB.tile([128, 1], f32, tag=f"fcol{dtv}")
                    nc.vector.tensor_scalar(out=ftile[:], in0=ff[:],
                                            scalar1=0.25, scalar2=0.125,
                                            op0=Alu.mult, op1=Alu.add)
                    fcol[dtv] = ftile
                gt = pB.tile([128, 3 * 2 * 256], f32, tag="gt")
                gt4 = gt[:].rearrange("p (c al w) -> p c al w", c=3, al=2)
                nc.sync.dma_start(
                    out=gt4, in_=g_d.rearrange("c (a al) w -> a c al w", al=2))
                pls = {}
                for al in range(2):
                    for be in range(2):
                        pl = pB.tile([128, 3 * 130], f16, tag=f"pl{al}{be}")
                        pl3 = pl[:].rearrange("p (c w) -> p c w", c=3)
                        nc.gpsimd.memset(pl3[:, :, 0:130:129], 0.0)
                        nc.scalar.copy(out=pl3[:, :, 1:129],
                                       in_=gt4[:, :, al, be:be + 255:2])
                        pls[(al, be)] = pl3
                pl00s = {0: pls[(0, 0)]}
                for da in (-1, 1):
                    t_ = pB.tile([128, 3 * 130], f16, tag=f"pl00s{da}")
                    t3 = t_[:].rearrange("p (c w) -> p c w", c=3)
                    nc.gpsimd.memset(t3, 0.0)
                    p0, p1 = max(0, -da), 128 - max(0, da)
                    nc.sync.dma_start(out=t3[p0:p1], in_=pls[(0, 0)][p0 + da:p1 + da])
                    pl00s[da] = t3
                n2c = {}
                sq = pB.tile([128, 3 * 130], f16, tag="sq")
                sq3 = sq[:].rearrange("p (c w) -> p c w", c=3)
                for par, pl3 in pls.items():
                    n2 = pB.tile([128, 130], f16, tag=f"n2{par[0]}{par[1]}")
                    nc.vector.scalar_tensor_tensor(
                        out=sq3, in0=pl3, scalar=0.5, in1=pl3,
                        op0=Alu.mult, op1=Alu.mult)
                    nc.vector.tensor_reduce(
                        out=n2[:], in_=sq3.rearrange("p c w -> p w c"),
                        axis=mybir.AxisListType.X, op=Alu.add)
                    n2c[par] = n2
                n200s = {0: n2c[(0, 0)]}
                for da in (-1, 1):
                    t_ = pB.tile([128, 130], f16, tag=f"n200s{da}")
                    nc.gpsimd.memset(t_[:], 0.0)
                    p0, p1 = max(0, -da), 128 - max(0, da)
                    nc.sync.dma_start(out=t_[p0:p1], in_=n2c[(0, 0)][p0 + da:p1 + da])
                    n200s[da] = t_
                kern3 = kern[:].rearrange("p (t w) -> p t w", w=128)
                nc.scalar.mul(out=kern3[:, 12, :], in_=grow[0][:],
                              mul=fcol[0][:])
                with tc.tile_pool(name="phB2", bufs=3) as pB2:
                    for i in range(K):
                        for j in range(K):
                            if i == 2 and j == 2:
                                continue
                            al, be = i % 2, j % 2
                            gda = (al + i - 2) // 2
                            gdw = (be + j - 2) // 2
                            nb = pl00s[gda][:, :, 1 + gdw:129 + gdw]
                            ce = pls[(al, be)][:, :, 1:129]
                            pr = pB2.tile([128, 3 * 128], f16, tag="pr")
                            pr3 = pr[:].rearrange("p (c w) -> p c w", c=3)
                            nc.gpsimd.tensor_tensor(out=pr3, in0=nb, in1=ce,
                                                      op=Alu.mult)
                            dot = pB2.tile([128, 128], f16, tag="dot")
                            nc.vector.tensor_reduce(
                                out=dot[:], in_=pr3.rearrange("p c w -> p w c"),
                                axis=mybir.AxisListType.X, op=Alu.add)
                            nc.gpsimd.tensor_tensor(
                                out=dot[:],
                                in0=n200s[gda][:, 1 + gdw:129 + gdw],
                                in1=dot[:], op=Alu.subtract)
                            nc.gpsimd.tensor_tensor(
                                out=dot[:], in0=n2c[(al, be)][:, 1:129],
                                in1=dot[:], op=Alu.add)
                            tau = 5 * i + j
                            nc.scalar.activation(out=kern3[:, tau, :],
                                                 in_=dot[:], func=Act.Exp,
                                                 scale=-1.0)
                            dtv, dsv = _DT[i], _DT[j]
                            # x grow (GpSimd TT), then x fcol (ACT per-part
                            # scale) -- keeps these off the busy DVE
                            nc.gpsimd.tensor_tensor(
                                out=kern3[:, tau, :], in0=kern3[:, tau, :],
                                in1=grow[dsv][:], op=Alu.mult)
                            nc.scalar.mul(out=kern3[:, tau, :],
                                          in_=kern3[:, tau, :],
                                          mul=fcol[dtv][:])

            # ---- phase C ----
            ybuf = pers.tile([128, 25 * 64 * R], f16, tag="ybuf",
                             name="ybuf")
            ring5 = ybuf[:].rearrange("p (t o r) -> p t o r", t=25, o=64)
            ring6 = ybuf[:].rearrange("p (i j o r) -> p i j o r",
                                      i=5, j=5, o=64)
            kern5 = kern[:].rearrange("p (i j w) -> p i j w", i=5, j=5)
            bias_ap = bias16[:].rearrange("p (o one) -> p o one", one=1) \
                               .broadcast_to([128, 64, WB])
            ovw = o_d.rearrange("o (t a) wq -> a t o wq", a=2)

            with tc.tile_pool(name="psC", bufs=1, space="PSUM") as pps, \
                 tc.tile_pool(name="mC", bufs=1) as pm, \
                 tc.tile_pool(name="stgC", bufs=1) as pstg:

                P = [pps.tile([128, 2048], f32, tag=f"P{h}", name=f"P{h}")
                     for h in range(2)]
                pvs = [P[h][:, 0:1600].rearrange("p (c t o) -> p c t o",
                                                 c=2, o=32) for h in range(2)]

                def emit_block(b):
                    wp0 = WB * b
                    stg_t = pstg.tile([128, 4 * 64 * WB], f32, tag="stg")
                    # [al, o, w, be]: wq = 2w+be interleaved for contig DMA
                    stg5 = stg_t[:].rearrange("p (a o w be) -> p a o w be",
                                              a=2, o=64, be=2)
                    for ci, (al, be) in enumerate(
                            [(0, 0), (0, 1), (1, 0), (1, 1)]):
                        ni = 3 if al == 0 else 2
                        J = [j for j in range(K) if j % 2 == be]
                        nj = len(J)
                        nl = nj + 1
                        m_t = pm.tile([128, ni * nl * 64 * WB], f16,
                                      tag=f"m{al}{be}", name=f"m{al}{be}")
                        m5 = m_t[:].rearrange("p (i j o w) -> p i j o w",
                                              i=ni, j=nl, o=64)
                        for jx, jc in enumerate(J):
                            ds = (be + jc - 2) // 2
                            u0b = wp0 + 1 + ds
                            wo, left, uu = 0, WB, u0b
                            while left > 0:
                                s = uu % R
                                run = min(left, R - s)
                                in_r = ring6[:, al::2, jc, :, s:s + run]
                                in_k = kern5[:, al::2, jc,
                                             wp0 + wo:wp0 + wo + run] \
                                    .rearrange("p i (one w) -> p i one w",
                                               one=1) \
                                    .broadcast_to([128, ni, 64, run])
                                out_m = m5[:, :, jx, :, wo:wo + run]
                                nc.vector.tensor_tensor(out=out_m, in0=in_r,
                                                        in1=in_k, op=Alu.mult)
                                wo += run
                                uu += run
                                left -= run
                        # whole add-chain on ONE engine per class (al=0 ->
                        # DVE, al=1 -> GpSimd); every add writes a FRESH
                        # destination lane (in-place RMW defeats DVE 2x)
                        eng = nc.vector if al == 0 else nc.gpsimd
                        eng.tensor_tensor(
                            out=m5[:, :, nj, :, :], in0=m5[:, :, 0, :, :],
                            in1=m5[:, :, 1, :, :], op=Alu.add)
                        js = nj
                        if nj == 3:
                            eng.tensor_tensor(
                                out=m5[:, :, 0, :, :],
                                in0=m5[:, :, nj, :, :],
                                in1=m5[:, :, 2, :, :], op=Alu.add)
                            js = 0
                        eng.tensor_tensor(
                            out=m5[:, 0, 1, :, :], in0=m5[:, 0, js, :, :],
                            in1=m5[:, 1, js, :, :], op=Alu.add)
                        isrc = (0, 1)
                        if ni == 3:
                            eng.tensor_tensor(
                                out=m5[:, 1, 1, :, :], in0=m5[:, 0, 1, :, :],
                                in1=m5[:, 2, js, :, :], op=Alu.add)
                            isrc = (1, 1)
                        # final: + bias, fp32 out, be-interleaved write
                        eng.tensor_tensor(
                            out=stg5[:, al, :, :, be],
                            in0=m5[:, isrc[0], isrc[1], :, :],
                            in1=bias_ap, op=Alu.add)
                    for al in range(2):
                        nc.sync.dma_start(
                            out=ovw[al, :, :, 2 * wp0:2 * wp0 + 2 * WB],
                            in_=stg5[:, al].rearrange("p o w be -> p o (w be)"))

                # pre-zero the c=0 regions (pair 0 has pad column u=0)
                for h in range(2):
                    nc.vector.memset(P[h][:, 0:800], 0.0)

                for k in range(65):
                    u0, u1 = 2 * k, 2 * k + 1
                    if k == 64:
                        # col 129 is pad: zero the c=1 regions
                        for h in range(2):
                            nc.vector.memset(P[h][:, 800:1600], 0.0)
                    for h in range(2):
                        for c, u in ((0, u0), (1, u1)):
                            if u in (0, 129):
                                continue
                            pu = 1 + 32 * ((u - 1) % 4) + (u - 1) // 4
                            for (t0, nt, dt) in _CH[c]:
                                nc.tensor.matmul(
                                    out=pvs[h][:, c, t0:t0 + nt, :],
                                    lhsT=xd3[:, 1 + dt:129 + dt, pu],
                                    rhs=wb3[:, t0:t0 + nt, 32 * h:32 * h + 32],
                                    start=True, stop=True)
                        # drain half-pair
                        s = u0 % R
                        dst = ring5[:, :, 32 * h:32 * h + 32, s:s + 2]
                        srcv = pvs[h].rearrange("p c t o -> p t o c")
                        if h == 1 and k % DVE_DRAIN_MOD == DVE_DRAIN_PHASE:
                            nc.vector.tensor_copy(dst, srcv)
                        else:
                            nc.scalar.copy(out=dst, in_=srcv)
                    if k >= 8 and k % 8 == 0:
                        emit_block(k // 8 - 1)
    nc.compile()
    return nc


_CACHE = {}


def _get_nc():
    if "nc" not in _CACHE:
        _CACHE["nc"] = _build_program()
    return _CACHE["nc"]


def kernel(x, guide, weight, bias, target_size=None, **_unused):
    from concourse.bass_utils import run_bass_kernel_spmd

    nc = _get_nc()
    x = np.ascontiguousarray(np.asarray(x, dtype=np.float32))
    guide = np.ascontiguousarray(np.asarray(guide, dtype=np.float32))
    weight = np.ascontiguousarray(np.asarray(weight, dtype=np.float32))
    bias = np.ascontiguousarray(np.asarray(bias, dtype=np.float32))
    in_maps = [
        {"x": x[b], "guide": guide[b], "weight": weight, "bias": bias}
        for b in range(B)
    ]
    res = run_bass_kernel_spmd(nc, in_maps, list(range(N_CORES))).results
    return np.stack([res[b]["out"] for b in range(B)], axis=0).astype(np.float32)


# revision 20
# speedup vs baseline: 1.0134x; 1.0134x over previous
"""Trainium2 Bass kernel for nn_DynamicUpsamplePAC (v2: pair-drain scheme).

Math (per batch item, data-parallel over B=8 -> 8 cores):
  1. x_d = bilinear downsample 160->128 (torch align_corners=False), fp16,
     held zero-padded as xd[c, 130, 130] (scale-deferred; undone in kern).
  2. kern[tau, t, w'] = exp(-0.5||g_nb - g_c||^2) at the 25 used
     (class, tap) combos; tau = 5i+j lex order; kern[12] == 1.
  3. Parity decomposition: out pixel (2t+al, 2w'+be) sums taps (i,j) with
     i%2==al, j%2==be reading x_d at (t+dt(i), w'+ds(j)),
     dt = [-1,0,0,1,1][i], ds = [-1,0,0,1,1][j].

Phase C v2 (the change vs v1):
  - Columns processed in PAIRS (u=2k,2k+1) with two single-buffered PSUM
    half-tiles P0/P1 = [pair(2), tau(25), o-half(32)] that ping-pong:
    while ACT drains P0, PE matmuls fill P1.  Drains write the ring
    ybuf[t?au, o, slot] with an innermost slot-PAIR (stride-1 run of 2
    fp16 = 4B), which measures ~1.26 ns/elem vs 4.6 ns/elem for the v1
    single-slot (stride-26) scatter that made ScalarE 88% busy.
  - WB=16 w'-blocks (8 blocks): DVE mults (2x fp16, w-innermost, kern
    o-broadcast middle), tap-sum adds on GpSimd, final i-add + bias on
    DVE writing fp32 directly, one output DMA per parity class with a
    strided DRAM view (be-interleave done by the DMA, not GpSimd).
"""

import numpy as np

B, C_IN, C_OUT, K = 8, 128, 64, 5
H_IN = W_IN = 160
T = 256
N_CORES = 8
R = 24          # ybuf ring slots (even; >= WB + 8)
WB = 16         # w'-block width
NBLK = 128 // WB

_FRACS = [0.125, 0.375, 0.625, 0.875]
_DT = [-1, 0, 0, 1, 1]

# matmul chunks per pair-column position: (tau0, ntau, dt), each chunk
# within one 512-fp32 PSUM bank given the half-tile base (c*800).
_CH = [
    [(0, 5, -1), (5, 10, 0), (15, 1, 1), (16, 9, 1)],             # c=0
    [(0, 5, -1), (5, 2, 0), (7, 8, 0), (15, 8, 1), (23, 2, 1)],   # c=1
]

DVE_DRAIN_MOD = 999   # pairs k with k%MOD==PHASE get h=1 drained on DVE
DVE_DRAIN_PHASE = 3


def _build_program(reps=1):
    import concourse.bass as bass  # noqa: F401
    import concourse.mybir as mybir
    import concourse.tile as tile
    from concourse import bacc

    f32 = mybir.dt.float32
    f16 = mybir.dt.float16
    i32 = mybir.dt.int32
    Alu = mybir.AluOpType
    Act = mybir.ActivationFunctionType

    nc = bacc.Bacc("TRN2", target_bir_lowering=False, debug=False,
                   enable_asserts=False)

    x_d = nc.dram_tensor("x", [C_IN, H_IN, W_IN], f32, kind="ExternalInput").ap()
    g_d = nc.dram_tensor("guide", [3, T, T], f32, kind="ExternalInput").ap()
    w_d = nc.dram_tensor("weight", [C_IN, C_OUT, K, K], f32, kind="ExternalInput").ap()
    b_d = nc.dram_tensor("bias", [C_OUT], f32, kind="ExternalInput").ap()
    o_d = nc.dram_tensor("out", [C_OUT, T, T], f32, kind="ExternalOutput").ap()

    with tile.TileContext(nc) as tc:
      for _rep in range(reps):
        with tc.tile_pool(name="pers", bufs=1) as pers, \
             nc.allow_low_precision("fp16 pipeline, fp32 dyn range ok"):
            xd = pers.tile([128, 130 * 130], f16, tag="xd")
            xd3 = xd[:].rearrange("p (v s) -> p v s", s=130)
            kern = pers.tile([128, 25 * 128], f16, tag="kern")
            wb = pers.tile([128, 25 * 64], f16, tag="wb")
            wb3 = wb[:].rearrange("p (t o) -> p t o", o=64)
            bias16 = pers.tile([128, 64], f16, tag="bias16")
            # ybuf is allocated after phases A/B (lifetime-disjoint with
            # their big scratch) -- see phase C below.

            # ---- weights: contig DMA + one ACT reorder/cast ----
            with tc.tile_pool(name="wld", bufs=1) as pw:
                wst = pw.tile([128, 1600], f32, tag="wst")
                nc.sync.dma_start(out=wst[:],
                                  in_=w_d.rearrange("c o i j -> c (o i j)"))
                wst4 = wst[:].rearrange("p (o i j) -> p o i j", i=K, j=K)
                wb4 = wb[:].rearrange("p (i j o) -> p i j o", i=K, j=K)
                nc.scalar.copy(out=wb4, in_=wst4.rearrange("p o i j -> p i j o"))

                bias_bc = pw.tile([128, 64], f32, tag="bias_bc")
                b_src = b_d.rearrange("(o a) -> o a", a=1) \
                           .broadcast_to([64, 128]).rearrange("o r -> r o")
                nc.sync.dma_start(out=bias_bc[:], in_=b_src)
                nc.vector.tensor_copy(bias16[:], bias_bc[:])

            # ---- phase A: bilinear downsample -> xd fp16, scale-deferred --
            # xd[c, row, phys] = true/(f(r)*g(rw)); cols rw-major:
            # phys(u) = 1 + 32*rw + m for data col u-1 = 4m+rw; pads at 0/129.
            # The deferred 1/(f*g) is folded into kern in phase B.
            nc.gpsimd.memset(xd3[:, 0, :], 0.0)
            nc.gpsimd.memset(xd3[:, 129, :], 0.0)
            nc.gpsimd.memset(xd3[:, :, 0], 0.0)
            nc.gpsimd.memset(xd3[:, :, 129], 0.0)
            with tc.tile_pool(name="phAw", bufs=1) as pAw, \
                 tc.tile_pool(name="phA", bufs=2) as pA:
                xw = pAw.tile([128, 160 * 128], f16, tag="xw")
                xw4 = xw[:].rearrange("p (r rw m) -> p r rw m", rw=4, m=32)
                for ch in range(5):
                    xh = pA.tile([128, 32 * 160], f32, tag="xh")
                    xh3 = xh[:].rearrange("p (r w) -> p r w", w=160)
                    nc.sync.dma_start(out=xh3,
                                      in_=x_d[:, 32 * ch:32 * ch + 32, :])
                    for rw in range(4):
                        g_rw = _FRACS[rw]
                        c2 = (1.0 - g_rw) / g_rw
                        aw = xh3[:, :, rw:rw + 156:5]
                        bw = xh3[:, :, rw + 1:rw + 157:5]
                        nc.vector.scalar_tensor_tensor(
                            out=xw4[:, 32 * ch:32 * ch + 32, rw, :],
                            in0=aw, scalar=c2, in1=bw,
                            op0=Alu.mult, op1=Alu.add)
                xw3 = xw[:].rearrange("p (r w) -> p r w", w=128)
                for r in range(4):
                    c_r = (1.0 - _FRACS[r]) / _FRACS[r]
                    a_v = xw3[:, r:r + 156:5, :]
                    b_v = xw3[:, r + 1:r + 157:5, :]
                    dst = xd3[:, 1 + r:1 + r + 125:4, 1:129]
                    nc.vector.scalar_tensor_tensor(
                        out=dst, in0=a_v, scalar=c_r, in1=b_v,
                        op0=Alu.mult, op1=Alu.add)

            # ---- phase B: pac kernel -> kern [t, tau, 128] fp16 ----
            # kern is pre-scaled by f((t+dt)%4)*g((wp+ds)%4) to undo the
            # deferred bilinear scales living in xd.
            with tc.tile_pool(name="phB", bufs=1) as pB:
                grow = {}
                for dsv in (-1, 0, 1):
                    gtile = pB.tile([128, 128], f16, tag=f"grow{dsv}")
                    g3 = gtile[:].rearrange("p (m k) -> p m k", k=4)
                    for k in range(4):
                        nc.gpsimd.memset(g3[:, :, k],
                                         float(_FRACS[(k + dsv) % 4]))
                    grow[dsv] = gtile
                fcol = {}
                fi = pB.tile([128, 1], i32, tag="fi")
                ff = pB.tile([128, 1], f32, tag="ff")
                for dtv in (-1, 0, 1):
                    nc.gpsimd.iota(fi[:], pattern=[[0, 1]], base=dtv + 4,
                                   channel_multiplier=1)
                    nc.vector.tensor_scalar(out=fi[:], in0=fi[:], scalar1=3,
                                            scalar2=None,
                                            op0=Alu.bitwise_and)
                    nc.vector.tensor_copy(ff[:], fi[:])
                    ftile = pB.tile([128, 1], f32, tag=f"fcol{dtv}")
                    nc.vector.tensor_scalar(out=ftile[:], in0=ff[:],
                                            scalar1=0.25, scalar2=0.125,
                                            op0=Alu.mult, op1=Alu.add)
                    fcol[dtv] = ftile
                gt = pB.tile([128, 3 * 2 * 256], f32, tag="gt")
                gt4 = gt[:].rearrange("p (c al w) -> p c al w", c=3, al=2)
                nc.sync.dma_start(
                    out=gt4, in_=g_d.rearrange("c (a al) w -> a c al w", al=2))
                pls = {}
                for al in range(2):
                    for be in range(2):
                        pl = pB.tile([128, 3 * 130], f16, tag=f"pl{al}{be}")
                        pl3 = pl[:].rearrange("p (c w) -> p c w", c=3)
                        nc.gpsimd.memset(pl3[:, :, 0:130:129], 0.0)
                        nc.scalar.copy(out=pl3[:, :, 1:129],
                                       in_=gt4[:, :, al, be:be + 255:2])
                        pls[(al, be)] = pl3
                pl00s = {0: pls[(0, 0)]}
                for da in (-1, 1):
                    t_ = pB.tile([128, 3 * 130], f16, tag=f"pl00s{da}")
                    t3 = t_[:].rearrange("p (c w) -> p c w", c=3)
                    nc.gpsimd.memset(t3, 0.0)
                    p0, p1 = max(0, -da), 128 - max(0, da)
                    nc.sync.dma_start(out=t3[p0:p1], in_=pls[(0, 0)][p0 + da:p1 + da])
                    pl00s[da] = t3
                n2c = {}
                sq = pB.tile([128, 3 * 130], f16, tag="sq")
                sq3 = sq[:].rearrange("p (c w) -> p c w", c=3)
                for par, pl3 in pls.items():
                    n2 = pB.tile([128, 130], f16, tag=f"n2{par[0]}{par[1]}")
                    nc.vector.scalar_tensor_tensor(
                        out=sq3, in0=pl3, scalar=0.5, in1=pl3,
                        op0=Alu.mult, op1=Alu.mult)
                    nc.vector.tensor_reduce(
                        out=n2[:], in_=sq3.rearrange("p c w -> p w c"),
                        axis=mybir.AxisListType.X, op=Alu.add)
                    n2c[par] = n2
                n200s = {0: n2c[(0, 0)]}
                for da in (-1, 1):
                    t_ = pB.tile([128, 130], f16, tag=f"n200s{da}")
                    nc.gpsimd.memset(t_[:], 0.0)
                    p0, p1 = max(0, -da), 128 - max(0, da)
                    nc.sync.dma_start(out=t_[p0:p1], in_=n2c[(0, 0)][p0 + da:p1 + da])
                    n200s[da] = t_
                kern3 = kern[:].rearrange("p (t w) -> p t w", w=128)
                nc.scalar.mul(out=kern3[:, 12, :], in_=grow[0][:],
                              mul=fcol[0][:])
                with tc.tile_pool(name="phB2", bufs=3) as pB2:
                    for i in range(K):
                        for j in range(K):
                            if i == 2 and j == 2:
                                continue
                            al, be = i % 2, j % 2
                            gda = (al + i - 2) // 2
                            gdw = (be + j - 2) // 2
                            nb = pl00s[gda][:, :, 1 + gdw:129 + gdw]
                            ce = pls[(al, be)][:, :, 1:129]
                            pr = pB2.tile([128, 3 * 128], f16, tag="pr")
                            pr3 = pr[:].rearrange("p (c w) -> p c w", c=3)
                            nc.gpsimd.tensor_tensor(out=pr3, in0=nb, in1=ce,
                                                      op=Alu.mult)
                            dot = pB2.tile([128, 128], f16, tag="dot")
                            nc.vector.tensor_reduce(
                                out=dot[:], in_=pr3.rearrange("p c w -> p w c"),
                                axis=mybir.AxisListType.X, op=Alu.add)
                            nc.gpsimd.tensor_tensor(
                                out=dot[:],
                                in0=n200s[gda][:, 1 + gdw:129 + gdw],
                                in1=dot[:], op=Alu.subtract)
                            nc.gpsimd.tensor_tensor(
                                out=dot[:], in0=n2c[(al, be)][:, 1:129],
                                in1=dot[:], op=Alu.add)
                            tau = 5 * i + j
                            nc.scalar.activation(out=kern3[:, tau, :],
                                                 in_=dot[:], func=Act.Exp,
                                                 scale=-1.0)
                            dtv, dsv = _DT[i], _DT[j]
                            # x grow (GpSimd TT), then x fcol (ACT per-part
                            # scale) -- keeps these off the busy DVE
                            nc.gpsimd.tensor_tensor(
                                out=kern3[:, tau, :], in0=kern3[:, tau, :],
                                in1=grow[dsv][:], op=Alu.mult)
                            nc.scalar.mul(out=kern3[:, tau, :],
                                          in_=kern3[:, tau, :],
                                          mul=fcol[dtv][:])

            # ---- phase C ----
            ybuf = pers.tile([128, 25 * 64 * R], f16, tag="ybuf",
                             name="ybuf")
            ring5 = ybuf[:].rearrange("p (t o r) -> p t o r", t=25, o=64)
            ring6 = ybuf[:].rearrange("p (i j o r) -> p i j o r",
                                      i=5, j=5, o=64)
            kern5 = kern[:].rearrange("p (i j w) -> p i j w", i=5, j=5)
            bias_ap = bias16[:].rearrange("p (o one) -> p o one", one=1) \
                               .broadcast_to([128, 64, WB])
            ovw = o_d.rearrange("o (t a) wq -> a t o wq", a=2)

            with tc.tile_pool(name="psC", bufs=1, space="PSUM") as pps, \
                 tc.tile_pool(name="mC", bufs=1) as pm, \
                 tc.tile_pool(name="stgC", bufs=2) as pstg:

                P = [pps.tile([128, 2048], f32, tag=f"P{h}", name=f"P{h}")
                     for h in range(2)]
                pvs = [P[h][:, 0:1600].rearrange("p (c t o) -> p c t o",
                                                 c=2, o=32) for h in range(2)]

                def emit_block(b):
                    wp0 = WB * b
                    stg_t = pstg.tile([128, 4 * 64 * WB], f32, tag="stg")
                    # [al, o, w, be]: wq = 2w+be interleaved for contig DMA
                    stg5 = stg_t[:].rearrange("p (a o w be) -> p a o w be",
                                              a=2, o=64, be=2)
                    for ci, (al, be) in enumerate(
                            [(0, 0), (0, 1), (1, 0), (1, 1)]):
                        ni = 3 if al == 0 else 2
                        J = [j for j in range(K) if j % 2 == be]
                        nj = len(J)
                        m_t = pm.tile([128, ni * nj * 64 * WB], f16,
                                      tag=f"m{al}{be}", name=f"m{al}{be}")
                        m5 = m_t[:].rearrange("p (i j o w) -> p i j o w",
                                              i=ni, j=nj, o=64)
                        for jx, jc in enumerate(J):
                            ds = (be + jc - 2) // 2
                            u0b = wp0 + 1 + ds
                            wo, left, uu = 0, WB, u0b
                            while left > 0:
                                s = uu % R
                                run = min(left, R - s)
                                in_r = ring6[:, al::2, jc, :, s:s + run]
                                in_k = kern5[:, al::2, jc,
                                             wp0 + wo:wp0 + wo + run] \
                                    .rearrange("p i (one w) -> p i one w",
                                               one=1) \
                                    .broadcast_to([128, ni, 64, run])
                                out_m = m5[:, :, jx, :, wo:wo + run]
                                if ds == 0:
                                    i0, i1 = in_k, in_r
                                else:
                                    i0, i1 = in_r, in_k
                                nc.vector.tensor_tensor(out=out_m, in0=i0,
                                                        in1=i1, op=Alu.mult)
                                wo += run
                                uu += run
                                left -= run
                        # whole add-chain on ONE engine per class to avoid
                        # cross-engine semaphore ping-pong: al=0 -> DVE,
                        # al=1 -> GpSimd
                        eng = nc.vector if al == 0 else nc.gpsimd
                        eng.tensor_tensor(
                            out=m5[:, :, 0, :, :], in0=m5[:, :, 0, :, :],
                            in1=m5[:, :, 1, :, :], op=Alu.add)
                        if nj == 3:
                            eng.tensor_tensor(
                                out=m5[:, :, 0, :, :],
                                in0=m5[:, :, 0, :, :],
                                in1=m5[:, :, 2, :, :], op=Alu.add)
                        eng.tensor_tensor(
                            out=m5[:, 0, 0, :, :], in0=m5[:, 0, 0, :, :],
                            in1=m5[:, 1, 0, :, :], op=Alu.add)
                        if ni == 3:
                            eng.tensor_tensor(
                                out=m5[:, 0, 0, :, :], in0=m5[:, 0, 0, :, :],
                                in1=m5[:, 2, 0, :, :], op=Alu.add)
                        # final: + bias, fp32 out, be-interleaved write
                        eng.tensor_tensor(
                            out=stg5[:, al, :, :, be], in0=m5[:, 0, 0, :, :],
                            in1=bias_ap, op=Alu.add)
                    for al in range(2):
                        nc.sync.dma_start(
                            out=ovw[al, :, :, 2 * wp0:2 * wp0 + 2 * WB],
                            in_=stg5[:, al].rearrange("p o w be -> p o (w be)"))

                # pre-zero the c=0 regions (pair 0 has pad column u=0)
                for h in range(2):
                    nc.vector.memset(P[h][:, 0:800], 0.0)

                for k in range(65):
                    u0, u1 = 2 * k, 2 * k + 1
                    if k == 64:
                        # col 129 is pad: zero the c=1 regions
                        for h in range(2):
                            nc.vector.memset(P[h][:, 800:1600], 0.0)
                    for h in range(2):
                        for c, u in ((0, u0), (1, u1)):
                            if u in (0, 129):
                                continue
                            pu = 1 + 32 * ((u - 1) % 4) + (u - 1) // 4
                            for (t0, nt, dt) in _CH[c]:
                                nc.tensor.matmul(
                                    out=pvs[h][:, c, t0:t0 + nt, :],
                                    lhsT=xd3[:, 1 + dt:129 + dt, pu],
                                    rhs=wb3[:, t0:t0 + nt, 32 * h:32 * h + 32],
                                    start=True, stop=True)
                        # drain half-pair
                        s = u0 % R
                        dst = ring5[:, :, 32 * h:32 * h + 32, s:s + 2]
                        srcv = pvs[h].rearrange("p c t o -> p t o c")
                        if h == 1 and k % DVE_DRAIN_MOD == DVE_DRAIN_PHASE:
                            nc.vector.tensor_copy(dst, srcv)
                        else:
                            nc.scalar.copy(out=dst, in_=srcv)
                    if k >= 8 and k % 8 == 0:
                        emit_block(k // 8 - 1)
    nc.compile()
    return nc


_CACHE = {}


def _get_nc():
    if "nc" not in _CACHE:
        _CACHE["nc"] = _build_program()
    return _CACHE["nc"]


def kernel(x, guide, weight, bias, target_size=None, **_unused):
    from concourse.bass_utils import run_bass_kernel_spmd

    nc = _get_nc()
    x = np.ascontiguousarray(np.asarray(x, dtype=np.float32))
    guide = np.ascontiguousarray(np.asarray(guide, dtype=np.float32))
    weight = np.ascontiguousarray(np.asarray(weight, dtype=np.float32))
    bias = np.ascontiguousarray(np.asarray(bias, dtype=np.float32))
    in_maps = [
        {"x": x[b], "guide": guide[b], "weight": weight, "bias": bias}
        for b in range(B)
    ]
    res = run_bass_kernel_spmd(nc, in_maps, list(range(N_CORES))).results
    return np.stack([res[b]["out"] for b in range(B)], axis=0).astype(np.float32)


# revision 21
# speedup vs baseline: 1.1061x; 1.0915x over previous
"""Trainium2 Bass kernel for nn_DynamicUpsamplePAC (v2: pair-drain scheme).

Math (per batch item, data-parallel over B=8 -> 8 cores):
  1. x_d = bilinear downsample 160->128 (torch align_corners=False), fp16,
     held zero-padded as xd[c, 130, 130] (scale-deferred; undone in kern).
  2. kern[tau, t, w'] = exp(-0.5||g_nb - g_c||^2) at the 25 used
     (class, tap) combos; tau = 5i+j lex order; kern[12] == 1.
  3. Parity decomposition: out pixel (2t+al, 2w'+be) sums taps (i,j) with
     i%2==al, j%2==be reading x_d at (t+dt(i), w'+ds(j)),
     dt = [-1,0,0,1,1][i], ds = [-1,0,0,1,1][j].

Phase C v2 (the change vs v1):
  - Columns processed in PAIRS (u=2k,2k+1) with two single-buffered PSUM
    half-tiles P0/P1 = [pair(2), tau(25), o-half(32)] that ping-pong:
    while ACT drains P0, PE matmuls fill P1.  Drains write the ring
    ybuf[t?au, o, slot] with an innermost slot-PAIR (stride-1 run of 2
    fp16 = 4B), which measures ~1.26 ns/elem vs 4.6 ns/elem for the v1
    single-slot (stride-26) scatter that made ScalarE 88% busy.
  - WB=16 w'-blocks (8 blocks): DVE mults (2x fp16, w-innermost, kern
    o-broadcast middle), tap-sum adds on GpSimd, final i-add + bias on
    DVE writing fp32 directly, one output DMA per parity class with a
    strided DRAM view (be-interleave done by the DMA, not GpSimd).
"""

import numpy as np

B, C_IN, C_OUT, K = 8, 128, 64, 5
H_IN = W_IN = 160
T = 256
N_CORES = 8
R = 24          # ybuf ring slots (even; >= WB + 8)
WB = 16         # w'-block width
NBLK = 128 // WB

_FRACS = [0.125, 0.375, 0.625, 0.875]
_DT = [-1, 0, 0, 1, 1]

# matmul chunks per pair-column position: (tau0, ntau, dt), each chunk
# within one 512-fp32 PSUM bank given the half-tile base (c*800).
_CH = [
    [(0, 5, -1), (5, 10, 0), (15, 1, 1), (16, 9, 1)],             # c=0
    [(0, 5, -1), (5, 2, 0), (7, 8, 0), (15, 8, 1), (23, 2, 1)],   # c=1
]

DVE_DRAIN_MOD = 999   # pairs k with k%MOD==PHASE get h=1 drained on DVE
DVE_DRAIN_PHASE = 3


def _build_program(reps=1):
    import concourse.bass as bass  # noqa: F401
    import concourse.mybir as mybir
    import concourse.tile as tile
    from concourse import bacc

    f32 = mybir.dt.float32
    f16 = mybir.dt.float16
    i32 = mybir.dt.int32
    Alu = mybir.AluOpType
    Act = mybir.ActivationFunctionType

    nc = bacc.Bacc("TRN2", target_bir_lowering=False, debug=False,
                   enable_asserts=False)

    x_d = nc.dram_tensor("x", [C_IN, H_IN, W_IN], f32, kind="ExternalInput").ap()
    g_d = nc.dram_tensor("guide", [3, T, T], f32, kind="ExternalInput").ap()
    w_d = nc.dram_tensor("weight", [C_IN, C_OUT, K, K], f32, kind="ExternalInput").ap()
    b_d = nc.dram_tensor("bias", [C_OUT], f32, kind="ExternalInput").ap()
    o_d = nc.dram_tensor("out", [C_OUT, T, T], f32, kind="ExternalOutput").ap()

    with tile.TileContext(nc) as tc:
      for _rep in range(reps):
        with tc.tile_pool(name="pers", bufs=1) as pers, \
             nc.allow_low_precision("fp16 pipeline, fp32 dyn range ok"):
            xd = pers.tile([128, 130 * 130], f16, tag="xd")
            xd3 = xd[:].rearrange("p (v s) -> p v s", s=130)
            kern = pers.tile([128, 25 * 128], f16, tag="kern")
            wb = pers.tile([128, 25 * 64], f16, tag="wb")
            wb3 = wb[:].rearrange("p (t o) -> p t o", o=64)
            bias16 = pers.tile([128, 64], f16, tag="bias16")
            # ybuf is allocated after phases A/B (lifetime-disjoint with
            # their big scratch) -- see phase C below.

            # ---- weights: contig DMA + one ACT reorder/cast ----
            with tc.tile_pool(name="wld", bufs=1) as pw:
                wst = pw.tile([128, 1600], f32, tag="wst")
                nc.sync.dma_start(out=wst[:],
                                  in_=w_d.rearrange("c o i j -> c (o i j)"))
                wst4 = wst[:].rearrange("p (o i j) -> p o i j", i=K, j=K)
                wb4 = wb[:].rearrange("p (i j o) -> p i j o", i=K, j=K)
                nc.scalar.copy(out=wb4, in_=wst4.rearrange("p o i j -> p i j o"))

                bias_bc = pw.tile([128, 64], f32, tag="bias_bc")
                b_src = b_d.rearrange("(o a) -> o a", a=1) \
                           .broadcast_to([64, 128]).rearrange("o r -> r o")
                nc.sync.dma_start(out=bias_bc[:], in_=b_src)
                nc.vector.tensor_copy(bias16[:], bias_bc[:])

            # ---- phase A: bilinear downsample -> xd fp16, scale-deferred --
            # xd[c, row, phys] = true/(f(r)*g(rw)); cols rw-major:
            # phys(u) = 1 + 32*rw + m for data col u-1 = 4m+rw; pads at 0/129.
            # The deferred 1/(f*g) is folded into kern in phase B.
            nc.gpsimd.memset(xd3[:, 0, :], 0.0)
            nc.gpsimd.memset(xd3[:, 129, :], 0.0)
            nc.gpsimd.memset(xd3[:, :, 0], 0.0)
            nc.gpsimd.memset(xd3[:, :, 129], 0.0)
            with tc.tile_pool(name="phAw", bufs=1) as pAw, \
                 tc.tile_pool(name="phA", bufs=2) as pA:
                xw = pAw.tile([128, 160 * 128], f16, tag="xw")
                xw4 = xw[:].rearrange("p (r rw m) -> p r rw m", rw=4, m=32)
                for ch in range(5):
                    xh = pA.tile([128, 32 * 160], f32, tag="xh")
                    xh3 = xh[:].rearrange("p (r w) -> p r w", w=160)
                    nc.sync.dma_start(out=xh3,
                                      in_=x_d[:, 32 * ch:32 * ch + 32, :])
                    for rw in range(4):
                        g_rw = _FRACS[rw]
                        c2 = (1.0 - g_rw) / g_rw
                        aw = xh3[:, :, rw:rw + 156:5]
                        bw = xh3[:, :, rw + 1:rw + 157:5]
                        nc.vector.scalar_tensor_tensor(
                            out=xw4[:, 32 * ch:32 * ch + 32, rw, :],
                            in0=aw, scalar=c2, in1=bw,
                            op0=Alu.mult, op1=Alu.add)
                xw3 = xw[:].rearrange("p (r w) -> p r w", w=128)
                for r in range(4):
                    c_r = (1.0 - _FRACS[r]) / _FRACS[r]
                    a_v = xw3[:, r:r + 156:5, :]
                    b_v = xw3[:, r + 1:r + 157:5, :]
                    dst = xd3[:, 1 + r:1 + r + 125:4, 1:129]
                    nc.vector.scalar_tensor_tensor(
                        out=dst, in0=a_v, scalar=c_r, in1=b_v,
                        op0=Alu.mult, op1=Alu.add)

            # ---- phase B: pac kernel -> kern [t, tau, 128] fp16 ----
            # kern is pre-scaled by f((t+dt)%4)*g((wp+ds)%4) to undo the
            # deferred bilinear scales living in xd.
            with tc.tile_pool(name="phB", bufs=1) as pB:
                grow = {}
                for dsv in (-1, 0, 1):
                    gtile = pB.tile([128, 128], f16, tag=f"grow{dsv}")
                    g3 = gtile[:].rearrange("p (m k) -> p m k", k=4)
                    for k in range(4):
                        nc.gpsimd.memset(g3[:, :, k],
                                         float(_FRACS[(k + dsv) % 4]))
                    grow[dsv] = gtile
                fcol = {}
                fi = pB.tile([128, 1], i32, tag="fi")
                ff = pB.tile([128, 1], f32, tag="ff")
                for dtv in (-1, 0, 1):
                    nc.gpsimd.iota(fi[:], pattern=[[0, 1]], base=dtv + 4,
                                   channel_multiplier=1)
                    nc.vector.tensor_scalar(out=fi[:], in0=fi[:], scalar1=3,
                                            scalar2=None,
                                            op0=Alu.bitwise_and)
                    nc.vector.tensor_copy(ff[:], fi[:])
                    ftile = pB.tile([128, 1], f32, tag=f"fcol{dtv}")
                    nc.vector.tensor_scalar(out=ftile[:], in0=ff[:],
                                            scalar1=0.25, scalar2=0.125,
                                            op0=Alu.mult, op1=Alu.add)
                    fcol[dtv] = ftile
                gt = pB.tile([128, 3 * 2 * 256], f32, tag="gt")
                gt4 = gt[:].rearrange("p (c al w) -> p c al w", c=3, al=2)
                nc.sync.dma_start(
                    out=gt4, in_=g_d.rearrange("c (a al) w -> a c al w", al=2))
                pls = {}
                for al in range(2):
                    for be in range(2):
                        pl = pB.tile([128, 3 * 130], f16, tag=f"pl{al}{be}")
                        pl3 = pl[:].rearrange("p (c w) -> p c w", c=3)
                        nc.gpsimd.memset(pl3[:, :, 0:130:129], 0.0)
                        nc.scalar.copy(out=pl3[:, :, 1:129],
                                       in_=gt4[:, :, al, be:be + 255:2])
                        pls[(al, be)] = pl3
                pl00s = {0: pls[(0, 0)]}
                for da in (-1, 1):
                    t_ = pB.tile([128, 3 * 130], f16, tag=f"pl00s{da}")
                    t3 = t_[:].rearrange("p (c w) -> p c w", c=3)
                    nc.gpsimd.memset(t3, 0.0)
                    p0, p1 = max(0, -da), 128 - max(0, da)
                    nc.sync.dma_start(out=t3[p0:p1], in_=pls[(0, 0)][p0 + da:p1 + da])
                    pl00s[da] = t3
                n2c = {}
                sq = pB.tile([128, 3 * 130], f16, tag="sq")
                sq3 = sq[:].rearrange("p (c w) -> p c w", c=3)
                for par, pl3 in pls.items():
                    n2 = pB.tile([128, 130], f16, tag=f"n2{par[0]}{par[1]}")
                    nc.vector.scalar_tensor_tensor(
                        out=sq3, in0=pl3, scalar=0.5, in1=pl3,
                        op0=Alu.mult, op1=Alu.mult)
                    nc.vector.tensor_reduce(
                        out=n2[:], in_=sq3.rearrange("p c w -> p w c"),
                        axis=mybir.AxisListType.X, op=Alu.add)
                    n2c[par] = n2
                n200s = {0: n2c[(0, 0)]}
                for da in (-1, 1):
                    t_ = pB.tile([128, 130], f16, tag=f"n200s{da}")
                    nc.gpsimd.memset(t_[:], 0.0)
                    p0, p1 = max(0, -da), 128 - max(0, da)
                    nc.sync.dma_start(out=t_[p0:p1], in_=n2c[(0, 0)][p0 + da:p1 + da])
                    n200s[da] = t_
                kern3 = kern[:].rearrange("p (t w) -> p t w", w=128)
                nc.scalar.mul(out=kern3[:, 12, :], in_=grow[0][:],
                              mul=fcol[0][:])
                with tc.tile_pool(name="phB2", bufs=3) as pB2:
                    for i in range(K):
                        for j in range(K):
                            if i == 2 and j == 2:
                                continue
                            al, be = i % 2, j % 2
                            gda = (al + i - 2) // 2
                            gdw = (be + j - 2) // 2
                            nb = pl00s[gda][:, :, 1 + gdw:129 + gdw]
                            ce = pls[(al, be)][:, :, 1:129]
                            pr = pB2.tile([128, 3 * 128], f16, tag="pr")
                            pr3 = pr[:].rearrange("p (c w) -> p c w", c=3)
                            nc.gpsimd.tensor_tensor(out=pr3, in0=nb, in1=ce,
                                                      op=Alu.mult)
                            dot = pB2.tile([128, 128], f16, tag="dot")
                            nc.vector.tensor_reduce(
                                out=dot[:], in_=pr3.rearrange("p c w -> p w c"),
                                axis=mybir.AxisListType.X, op=Alu.add)
                            nc.gpsimd.tensor_tensor(
                                out=dot[:],
                                in0=n200s[gda][:, 1 + gdw:129 + gdw],
                                in1=dot[:], op=Alu.subtract)
                            nc.gpsimd.tensor_tensor(
                                out=dot[:], in0=n2c[(al, be)][:, 1:129],
                                in1=dot[:], op=Alu.add)
                            tau = 5 * i + j
                            nc.scalar.activation(out=kern3[:, tau, :],
                                                 in_=dot[:], func=Act.Exp,
                                                 scale=-1.0)
                            dtv, dsv = _DT[i], _DT[j]
                            # x grow (GpSimd TT), then x fcol (ACT per-part
                            # scale) -- keeps these off the busy DVE
                            nc.gpsimd.tensor_tensor(
                                out=kern3[:, tau, :], in0=kern3[:, tau, :],
                                in1=grow[dsv][:], op=Alu.mult)
                            nc.scalar.mul(out=kern3[:, tau, :],
                                          in_=kern3[:, tau, :],
                                          mul=fcol[dtv][:])

            # ---- phase C ----
            ybuf = pers.tile([128, 25 * 64 * R], f16, tag="ybuf",
                             name="ybuf")
            ring5 = ybuf[:].rearrange("p (t o r) -> p t o r", t=25, o=64)
            ring6 = ybuf[:].rearrange("p (i j o r) -> p i j o r",
                                      i=5, j=5, o=64)
            kern5 = kern[:].rearrange("p (i j w) -> p i j w", i=5, j=5)
            bias_ap = bias16[:].rearrange("p (o one) -> p o one", one=1) \
                               .broadcast_to([128, 64, WB])
            ovw = o_d.rearrange("o (t a) wq -> a t o wq", a=2)

            with tc.tile_pool(name="psC", bufs=1, space="PSUM") as pps, \
                 tc.tile_pool(name="mC", bufs=1) as pm, \
                 tc.tile_pool(name="stgC", bufs=2) as pstg:

                P = [pps.tile([128, 2048], f32, tag=f"P{h}", name=f"P{h}")
                     for h in range(2)]
                pvs = [P[h][:, 0:1600].rearrange("p (c t o) -> p c t o",
                                                 c=2, o=32) for h in range(2)]

                def emit_block(b):
                    wp0 = WB * b
                    stg_t = pstg.tile([128, 4 * 64 * WB], f32, tag="stg")
                    # [al, o, w, be]: wq = 2w+be interleaved for contig DMA
                    stg5 = stg_t[:].rearrange("p (a o w be) -> p a o w be",
                                              a=2, o=64, be=2)
                    for ci, (al, be) in enumerate(
                            [(0, 0), (0, 1), (1, 0), (1, 1)]):
                        ni = 3 if al == 0 else 2
                        J = [j for j in range(K) if j % 2 == be]
                        nj = len(J)
                        m_t = pm.tile([128, ni * nj * 64 * WB], f16,
                                      tag=f"m{al}{be}", name=f"m{al}{be}")
                        m5 = m_t[:].rearrange("p (i j o w) -> p i j o w",
                                              i=ni, j=nj, o=64)
                        for jx, jc in enumerate(J):
                            ds = (be + jc - 2) // 2
                            u0b = wp0 + 1 + ds
                            wo, left, uu = 0, WB, u0b
                            while left > 0:
                                s = uu % R
                                run = min(left, R - s)
                                in_r = ring6[:, al::2, jc, :, s:s + run]
                                in_k = kern5[:, al::2, jc,
                                             wp0 + wo:wp0 + wo + run] \
                                    .rearrange("p i (one w) -> p i one w",
                                               one=1) \
                                    .broadcast_to([128, ni, 64, run])
                                out_m = m5[:, :, jx, :, wo:wo + run]
                                if ds == 0:
                                    i0, i1 = in_k, in_r
                                else:
                                    i0, i1 = in_r, in_k
                                nc.vector.tensor_tensor(out=out_m, in0=i0,
                                                        in1=i1, op=Alu.mult)
                                wo += run
                                uu += run
                                left -= run
                        # big j-chain adds on DVE (GpSimd measured 6 ns/elem
                        # on these); only class (1,1)'s small chain + the
                        # i-chains/finals of al=1 stay on GpSimd
                        eng = nc.vector if al == 0 else nc.gpsimd
                        jeng = nc.gpsimd if (al, be) == (1, 1) else nc.vector
                        jeng.tensor_tensor(
                            out=m5[:, :, 0, :, :], in0=m5[:, :, 0, :, :],
                            in1=m5[:, :, 1, :, :], op=Alu.add)
                        if nj == 3:
                            jeng.tensor_tensor(
                                out=m5[:, :, 0, :, :],
                                in0=m5[:, :, 0, :, :],
                                in1=m5[:, :, 2, :, :], op=Alu.add)
                        eng.tensor_tensor(
                            out=m5[:, 0, 0, :, :], in0=m5[:, 0, 0, :, :],
                            in1=m5[:, 1, 0, :, :], op=Alu.add)
                        if ni == 3:
                            eng.tensor_tensor(
                                out=m5[:, 0, 0, :, :], in0=m5[:, 0, 0, :, :],
                                in1=m5[:, 2, 0, :, :], op=Alu.add)
                        # final: + bias, fp32 out, be-interleaved write
                        eng.tensor_tensor(
                            out=stg5[:, al, :, :, be], in0=m5[:, 0, 0, :, :],
                            in1=bias_ap, op=Alu.add)
                    for al in range(2):
                        nc.sync.dma_start(
                            out=ovw[al, :, :, 2 * wp0:2 * wp0 + 2 * WB],
                            in_=stg5[:, al].rearrange("p o w be -> p o (w be)"))

                # pre-zero the c=0 regions (pair 0 has pad column u=0)
                for h in range(2):
                    nc.vector.memset(P[h][:, 0:800], 0.0)

                for k in range(65):
                    u0, u1 = 2 * k, 2 * k + 1
                    if k == 64:
                        # col 129 is pad: zero the c=1 regions
                        for h in range(2):
                            nc.vector.memset(P[h][:, 800:1600], 0.0)
                    for h in range(2):
                        for c, u in ((0, u0), (1, u1)):
                            if u in (0, 129):
                                continue
                            pu = 1 + 32 * ((u - 1) % 4) + (u - 1) // 4
                            for (t0, nt, dt) in _CH[c]:
                                nc.tensor.matmul(
                                    out=pvs[h][:, c, t0:t0 + nt, :],
                                    lhsT=xd3[:, 1 + dt:129 + dt, pu],
                                    rhs=wb3[:, t0:t0 + nt, 32 * h:32 * h + 32],
                                    start=True, stop=True)
                        # drain half-pair
                        s = u0 % R
                        dst = ring5[:, :, 32 * h:32 * h + 32, s:s + 2]
                        srcv = pvs[h].rearrange("p c t o -> p t o c")
                        if h == 1 and k % DVE_DRAIN_MOD == DVE_DRAIN_PHASE:
                            nc.vector.tensor_copy(dst, srcv)
                        else:
                            nc.scalar.copy(out=dst, in_=srcv)
                    if k >= 8 and k % 8 == 0:
                        emit_block(k // 8 - 1)
    nc.compile()
    return nc


_CACHE = {}


def _get_nc():
    if "nc" not in _CACHE:
        _CACHE["nc"] = _build_program()
    return _CACHE["nc"]


def kernel(x, guide, weight, bias, target_size=None, **_unused):
    from concourse.bass_utils import run_bass_kernel_spmd

    nc = _get_nc()
    x = np.ascontiguousarray(np.asarray(x, dtype=np.float32))
    guide = np.ascontiguousarray(np.asarray(guide, dtype=np.float32))
    weight = np.ascontiguousarray(np.asarray(weight, dtype=np.float32))
    bias = np.ascontiguousarray(np.asarray(bias, dtype=np.float32))
    in_maps = [
        {"x": x[b], "guide": guide[b], "weight": weight, "bias": bias}
        for b in range(B)
    ]
    res = run_bass_kernel_spmd(nc, in_maps, list(range(N_CORES))).results
    return np.stack([res[b]["out"] for b in range(B)], axis=0).astype(np.float32)


# revision 22
# speedup vs baseline: 1.1602x; 1.0490x over previous
"""Trainium2 Bass kernel for nn_DynamicUpsamplePAC (v2: pair-drain scheme).

Math (per batch item, data-parallel over B=8 -> 8 cores):
  1. x_d = bilinear downsample 160->128 (torch align_corners=False), fp16,
     held zero-padded as xd[c, 130, 130] (scale-deferred; undone in kern).
  2. kern[tau, t, w'] = exp(-0.5||g_nb - g_c||^2) at the 25 used
     (class, tap) combos; tau = 5i+j lex order; kern[12] == 1.
  3. Parity decomposition: out pixel (2t+al, 2w'+be) sums taps (i,j) with
     i%2==al, j%2==be reading x_d at (t+dt(i), w'+ds(j)),
     dt = [-1,0,0,1,1][i], ds = [-1,0,0,1,1][j].

Phase C v2 (the change vs v1):
  - Columns processed in PAIRS (u=2k,2k+1) with two single-buffered PSUM
    half-tiles P0/P1 = [pair(2), tau(25), o-half(32)] that ping-pong:
    while ACT drains P0, PE matmuls fill P1.  Drains write the ring
    ybuf[t?au, o, slot] with an innermost slot-PAIR (stride-1 run of 2
    fp16 = 4B), which measures ~1.26 ns/elem vs 4.6 ns/elem for the v1
    single-slot (stride-26) scatter that made ScalarE 88% busy.
  - WB=16 w'-blocks (8 blocks): DVE mults (2x fp16, w-innermost, kern
    o-broadcast middle), tap-sum adds on GpSimd, final i-add + bias on
    DVE writing fp32 directly, one output DMA per parity class with a
    strided DRAM view (be-interleave done by the DMA, not GpSimd).
"""

import numpy as np

B, C_IN, C_OUT, K = 8, 128, 64, 5
H_IN = W_IN = 160
T = 256
N_CORES = 8
R = 24          # ybuf ring slots (even; >= WB + 8)
WB = 16         # w'-block width
NBLK = 128 // WB

_FRACS = [0.125, 0.375, 0.625, 0.875]
_DT = [-1, 0, 0, 1, 1]

# matmul chunks per pair-column position: (tau0, ntau, dt), each chunk
# within one 512-fp32 PSUM bank given the half-tile base (c*800).
_CH = [
    [(0, 5, -1), (5, 10, 0), (15, 1, 1), (16, 9, 1)],             # c=0
    [(0, 5, -1), (5, 2, 0), (7, 8, 0), (15, 8, 1), (23, 2, 1)],   # c=1
]

DVE_DRAIN_MOD = 999   # pairs k with k%MOD==PHASE get h=1 drained on DVE
DVE_DRAIN_PHASE = 3


def _build_program(reps=1):
    import concourse.bass as bass  # noqa: F401
    import concourse.mybir as mybir
    import concourse.tile as tile
    from concourse import bacc

    f32 = mybir.dt.float32
    f16 = mybir.dt.float16
    i32 = mybir.dt.int32
    Alu = mybir.AluOpType
    Act = mybir.ActivationFunctionType

    nc = bacc.Bacc("TRN2", target_bir_lowering=False, debug=False,
                   enable_asserts=False)

    x_d = nc.dram_tensor("x", [C_IN, H_IN, W_IN], f32, kind="ExternalInput").ap()
    g_d = nc.dram_tensor("guide", [3, T, T], f32, kind="ExternalInput").ap()
    w_d = nc.dram_tensor("weight", [C_IN, C_OUT, K, K], f32, kind="ExternalInput").ap()
    b_d = nc.dram_tensor("bias", [C_OUT], f32, kind="ExternalInput").ap()
    o_d = nc.dram_tensor("out", [C_OUT, T, T], f32, kind="ExternalOutput").ap()

    with tile.TileContext(nc) as tc:
      for _rep in range(reps):
        with tc.tile_pool(name="pers", bufs=1) as pers, \
             nc.allow_low_precision("fp16 pipeline, fp32 dyn range ok"):
            xd = pers.tile([128, 130 * 130], f16, tag="xd")
            xd3 = xd[:].rearrange("p (v s) -> p v s", s=130)
            kern = pers.tile([128, 25 * 128], f16, tag="kern")
            wb = pers.tile([128, 25 * 64], f16, tag="wb")
            wb3 = wb[:].rearrange("p (t o) -> p t o", o=64)
            bias16 = pers.tile([128, 64], f16, tag="bias16")
            # ybuf is allocated after phases A/B (lifetime-disjoint with
            # their big scratch) -- see phase C below.

            # ---- weights: contig DMA + one ACT reorder/cast ----
            with tc.tile_pool(name="wld", bufs=1) as pw:
                wst = pw.tile([128, 1600], f32, tag="wst")
                nc.sync.dma_start(out=wst[:],
                                  in_=w_d.rearrange("c o i j -> c (o i j)"))
                wst4 = wst[:].rearrange("p (o i j) -> p o i j", i=K, j=K)
                wb4 = wb[:].rearrange("p (i j o) -> p i j o", i=K, j=K)
                nc.scalar.copy(out=wb4, in_=wst4.rearrange("p o i j -> p i j o"))

                bias_bc = pw.tile([128, 64], f32, tag="bias_bc")
                b_src = b_d.rearrange("(o a) -> o a", a=1) \
                           .broadcast_to([64, 128]).rearrange("o r -> r o")
                nc.sync.dma_start(out=bias_bc[:], in_=b_src)
                nc.vector.tensor_copy(bias16[:], bias_bc[:])

            # ---- phase A: bilinear downsample -> xd fp16, scale-deferred --
            # xd[c, row, phys] = true/(f(r)*g(rw)); cols rw-major:
            # phys(u) = 1 + 32*rw + m for data col u-1 = 4m+rw; pads at 0/129.
            # The deferred 1/(f*g) is folded into kern in phase B.
            nc.gpsimd.memset(xd3[:, 0, :], 0.0)
            nc.gpsimd.memset(xd3[:, 129, :], 0.0)
            nc.gpsimd.memset(xd3[:, :, 0], 0.0)
            nc.gpsimd.memset(xd3[:, :, 129], 0.0)
            with tc.tile_pool(name="phAw", bufs=1) as pAw, \
                 tc.tile_pool(name="phA", bufs=2) as pA:
                xw = pAw.tile([128, 160 * 128], f16, tag="xw")
                xw4 = xw[:].rearrange("p (r rw m) -> p r rw m", rw=4, m=32)
                for ch in range(5):
                    xh = pA.tile([128, 32 * 160], f32, tag="xh")
                    xh3 = xh[:].rearrange("p (r w) -> p r w", w=160)
                    nc.sync.dma_start(out=xh3,
                                      in_=x_d[:, 32 * ch:32 * ch + 32, :])
                    for rw in range(4):
                        g_rw = _FRACS[rw]
                        c2 = (1.0 - g_rw) / g_rw
                        aw = xh3[:, :, rw:rw + 156:5]
                        bw = xh3[:, :, rw + 1:rw + 157:5]
                        nc.vector.scalar_tensor_tensor(
                            out=xw4[:, 32 * ch:32 * ch + 32, rw, :],
                            in0=aw, scalar=c2, in1=bw,
                            op0=Alu.mult, op1=Alu.add)
                xw3 = xw[:].rearrange("p (r w) -> p r w", w=128)
                for r in range(4):
                    c_r = (1.0 - _FRACS[r]) / _FRACS[r]
                    a_v = xw3[:, r:r + 156:5, :]
                    b_v = xw3[:, r + 1:r + 157:5, :]
                    dst = xd3[:, 1 + r:1 + r + 125:4, 1:129]
                    nc.vector.scalar_tensor_tensor(
                        out=dst, in0=a_v, scalar=c_r, in1=b_v,
                        op0=Alu.mult, op1=Alu.add)

            # ---- phase B: pac kernel -> kern [t, tau, 128] fp16 ----
            # kern is pre-scaled by f((t+dt)%4)*g((wp+ds)%4) to undo the
            # deferred bilinear scales living in xd.
            with tc.tile_pool(name="phB", bufs=1) as pB:
                grow = {}
                for dsv in (-1, 0, 1):
                    gtile = pB.tile([128, 128], f16, tag=f"grow{dsv}")
                    g3 = gtile[:].rearrange("p (m k) -> p m k", k=4)
                    for k in range(4):
                        nc.gpsimd.memset(g3[:, :, k],
                                         float(_FRACS[(k + dsv) % 4]))
                    grow[dsv] = gtile
                fcol = {}
                fi = pB.tile([128, 1], i32, tag="fi")
                ff = pB.tile([128, 1], f32, tag="ff")
                for dtv in (-1, 0, 1):
                    nc.gpsimd.iota(fi[:], pattern=[[0, 1]], base=dtv + 4,
                                   channel_multiplier=1)
                    nc.vector.tensor_scalar(out=fi[:], in0=fi[:], scalar1=3,
                                            scalar2=None,
                                            op0=Alu.bitwise_and)
                    nc.vector.tensor_copy(ff[:], fi[:])
                    ftile = pB.tile([128, 1], f32, tag=f"fcol{dtv}")
                    nc.vector.tensor_scalar(out=ftile[:], in0=ff[:],
                                            scalar1=0.25, scalar2=0.125,
                                            op0=Alu.mult, op1=Alu.add)
                    fcol[dtv] = ftile
                gt = pB.tile([128, 3 * 2 * 256], f32, tag="gt")
                gt4 = gt[:].rearrange("p (c al w) -> p c al w", c=3, al=2)
                nc.sync.dma_start(
                    out=gt4, in_=g_d.rearrange("c (a al) w -> a c al w", al=2))
                pls = {}
                for al in range(2):
                    for be in range(2):
                        pl = pB.tile([128, 3 * 130], f16, tag=f"pl{al}{be}")
                        pl3 = pl[:].rearrange("p (c w) -> p c w", c=3)
                        nc.gpsimd.memset(pl3[:, :, 0:130:129], 0.0)
                        nc.scalar.copy(out=pl3[:, :, 1:129],
                                       in_=gt4[:, :, al, be:be + 255:2])
                        pls[(al, be)] = pl3
                pl00s = {0: pls[(0, 0)]}
                for da in (-1, 1):
                    t_ = pB.tile([128, 3 * 130], f16, tag=f"pl00s{da}")
                    t3 = t_[:].rearrange("p (c w) -> p c w", c=3)
                    nc.gpsimd.memset(t3, 0.0)
                    p0, p1 = max(0, -da), 128 - max(0, da)
                    nc.sync.dma_start(out=t3[p0:p1], in_=pls[(0, 0)][p0 + da:p1 + da])
                    pl00s[da] = t3
                n2c = {}
                sq = pB.tile([128, 3 * 130], f16, tag="sq")
                sq3 = sq[:].rearrange("p (c w) -> p c w", c=3)
                for par, pl3 in pls.items():
                    n2 = pB.tile([128, 130], f16, tag=f"n2{par[0]}{par[1]}")
                    nc.vector.scalar_tensor_tensor(
                        out=sq3, in0=pl3, scalar=0.5, in1=pl3,
                        op0=Alu.mult, op1=Alu.mult)
                    nc.vector.tensor_reduce(
                        out=n2[:], in_=sq3.rearrange("p c w -> p w c"),
                        axis=mybir.AxisListType.X, op=Alu.add)
                    n2c[par] = n2
                n200s = {0: n2c[(0, 0)]}
                for da in (-1, 1):
                    t_ = pB.tile([128, 130], f16, tag=f"n200s{da}")
                    nc.gpsimd.memset(t_[:], 0.0)
                    p0, p1 = max(0, -da), 128 - max(0, da)
                    nc.sync.dma_start(out=t_[p0:p1], in_=n2c[(0, 0)][p0 + da:p1 + da])
                    n200s[da] = t_
                kern3 = kern[:].rearrange("p (t w) -> p t w", w=128)
                nc.scalar.mul(out=kern3[:, 12, :], in_=grow[0][:],
                              mul=fcol[0][:])
                with tc.tile_pool(name="phB2", bufs=3) as pB2:
                    for i in range(K):
                        for j in range(K):
                            if i == 2 and j == 2:
                                continue
                            al, be = i % 2, j % 2
                            gda = (al + i - 2) // 2
                            gdw = (be + j - 2) // 2
                            nb = pl00s[gda][:, :, 1 + gdw:129 + gdw]
                            ce = pls[(al, be)][:, :, 1:129]
                            pr = pB2.tile([128, 3 * 128], f16, tag="pr")
                            pr3 = pr[:].rearrange("p (c w) -> p c w", c=3)
                            nc.gpsimd.tensor_tensor(out=pr3, in0=nb, in1=ce,
                                                      op=Alu.mult)
                            dot = pB2.tile([128, 128], f16, tag="dot")
                            nc.vector.tensor_reduce(
                                out=dot[:], in_=pr3.rearrange("p c w -> p w c"),
                                axis=mybir.AxisListType.X, op=Alu.add)
                            nc.gpsimd.tensor_tensor(
                                out=dot[:],
                                in0=n200s[gda][:, 1 + gdw:129 + gdw],
                                in1=dot[:], op=Alu.subtract)
                            nc.gpsimd.tensor_tensor(
                                out=dot[:], in0=n2c[(al, be)][:, 1:129],
                                in1=dot[:], op=Alu.add)
                            tau = 5 * i + j
                            nc.scalar.activation(out=kern3[:, tau, :],
                                                 in_=dot[:], func=Act.Exp,
                                                 scale=-1.0)
                            dtv, dsv = _DT[i], _DT[j]
                            # x grow (GpSimd TT), then x fcol (ACT per-part
                            # scale) -- keeps these off the busy DVE
                            nc.gpsimd.tensor_tensor(
                                out=kern3[:, tau, :], in0=kern3[:, tau, :],
                                in1=grow[dsv][:], op=Alu.mult)
                            nc.scalar.mul(out=kern3[:, tau, :],
                                          in_=kern3[:, tau, :],
                                          mul=fcol[dtv][:])

            # ---- phase C ----
            ybuf = pers.tile([128, 25 * 64 * R], f16, tag="ybuf",
                             name="ybuf")
            ring5 = ybuf[:].rearrange("p (t o r) -> p t o r", t=25, o=64)
            ring6 = ybuf[:].rearrange("p (i j o r) -> p i j o r",
                                      i=5, j=5, o=64)
            kern5 = kern[:].rearrange("p (i j w) -> p i j w", i=5, j=5)
            bias_ap = bias16[:].rearrange("p (o one) -> p o one", one=1) \
                               .broadcast_to([128, 64, WB])
            ovw = o_d.rearrange("o (t a) wq -> a t o wq", a=2)

            with tc.tile_pool(name="psC", bufs=1, space="PSUM") as pps, \
                 tc.tile_pool(name="mC", bufs=1) as pm, \
                 tc.tile_pool(name="stgC", bufs=2) as pstg:

                P = [pps.tile([128, 2048], f32, tag=f"P{h}", name=f"P{h}")
                     for h in range(2)]
                pvs = [P[h][:, 0:1600].rearrange("p (c t o) -> p c t o",
                                                 c=2, o=32) for h in range(2)]

                def emit_block(b):
                    wp0 = WB * b
                    stg_t = pstg.tile([128, 4 * 64 * WB], f32, tag="stg")
                    # [al, o, w, be]: wq = 2w+be interleaved for contig DMA
                    stg5 = stg_t[:].rearrange("p (a o w be) -> p a o w be",
                                              a=2, o=64, be=2)
                    for ci, (al, be) in enumerate(
                            [(0, 0), (0, 1), (1, 0), (1, 1)]):
                        ni = 3 if al == 0 else 2
                        J = [j for j in range(K) if j % 2 == be]
                        nj = len(J)
                        m_t = pm.tile([128, ni * nj * 64 * WB], f16,
                                      tag=f"m{al}{be}", name=f"m{al}{be}")
                        m5 = m_t[:].rearrange("p (i j o w) -> p i j o w",
                                              i=ni, j=nj, o=64)
                        for jx, jc in enumerate(J):
                            ds = (be + jc - 2) // 2
                            u0b = wp0 + 1 + ds
                            wo, left, uu = 0, WB, u0b
                            while left > 0:
                                s = uu % R
                                run = min(left, R - s)
                                in_r = ring6[:, al::2, jc, :, s:s + run]
                                in_k = kern5[:, al::2, jc,
                                             wp0 + wo:wp0 + wo + run] \
                                    .rearrange("p i (one w) -> p i one w",
                                               one=1) \
                                    .broadcast_to([128, ni, 64, run])
                                out_m = m5[:, :, jx, :, wo:wo + run]
                                if ds == 0:
                                    i0, i1 = in_k, in_r
                                else:
                                    i0, i1 = in_r, in_k
                                nc.vector.tensor_tensor(out=out_m, in0=i0,
                                                        in1=i1, op=Alu.mult)
                                wo += run
                                uu += run
                                left -= run
                        # big j-chain adds on DVE (GpSimd measured 6 ns/elem
                        # on these); only class (1,1)'s small chain + the
                        # i-chains/finals of al=1 stay on GpSimd
                        eng = nc.vector if al == 0 else nc.gpsimd
                        jeng = nc.vector
                        jeng.tensor_tensor(
                            out=m5[:, :, 0, :, :], in0=m5[:, :, 0, :, :],
                            in1=m5[:, :, 1, :, :], op=Alu.add)
                        if nj == 3:
                            jeng.tensor_tensor(
                                out=m5[:, :, 0, :, :],
                                in0=m5[:, :, 0, :, :],
                                in1=m5[:, :, 2, :, :], op=Alu.add)
                        eng.tensor_tensor(
                            out=m5[:, 0, 0, :, :], in0=m5[:, 0, 0, :, :],
                            in1=m5[:, 1, 0, :, :], op=Alu.add)
                        if ni == 3:
                            eng.tensor_tensor(
                                out=m5[:, 0, 0, :, :], in0=m5[:, 0, 0, :, :],
                                in1=m5[:, 2, 0, :, :], op=Alu.add)
                        # final: + bias, fp32 out, be-interleaved write
                        eng.tensor_tensor(
                            out=stg5[:, al, :, :, be], in0=m5[:, 0, 0, :, :],
                            in1=bias_ap, op=Alu.add)
                    for al in range(2):
                        nc.sync.dma_start(
                            out=ovw[al, :, :, 2 * wp0:2 * wp0 + 2 * WB],
                            in_=stg5[:, al].rearrange("p o w be -> p o (w be)"))

                # pre-zero the c=0 regions (pair 0 has pad column u=0)
                for h in range(2):
                    nc.vector.memset(P[h][:, 0:800], 0.0)

                for k in range(65):
                    u0, u1 = 2 * k, 2 * k + 1
                    if k == 64:
                        # col 129 is pad: zero the c=1 regions
                        for h in range(2):
                            nc.vector.memset(P[h][:, 800:1600], 0.0)
                    for h in range(2):
                        for c, u in ((0, u0), (1, u1)):
                            if u in (0, 129):
                                continue
                            pu = 1 + 32 * ((u - 1) % 4) + (u - 1) // 4
                            for (t0, nt, dt) in _CH[c]:
                                nc.tensor.matmul(
                                    out=pvs[h][:, c, t0:t0 + nt, :],
                                    lhsT=xd3[:, 1 + dt:129 + dt, pu],
                                    rhs=wb3[:, t0:t0 + nt, 32 * h:32 * h + 32],
                                    start=True, stop=True)
                        # drain half-pair
                        s = u0 % R
                        dst = ring5[:, :, 32 * h:32 * h + 32, s:s + 2]
                        srcv = pvs[h].rearrange("p c t o -> p t o c")
                        if h == 1 and k % DVE_DRAIN_MOD == DVE_DRAIN_PHASE:
                            nc.vector.tensor_copy(dst, srcv)
                        else:
                            nc.scalar.copy(out=dst, in_=srcv)
                    if k >= 8 and k % 8 == 0:
                        emit_block(k // 8 - 1)
    nc.compile()
    return nc


_CACHE = {}


def _get_nc():
    if "nc" not in _CACHE:
        _CACHE["nc"] = _build_program()
    return _CACHE["nc"]


def kernel(x, guide, weight, bias, target_size=None, **_unused):
    from concourse.bass_utils import run_bass_kernel_spmd

    nc = _get_nc()
    x = np.ascontiguousarray(np.asarray(x, dtype=np.float32))
    guide = np.ascontiguousarray(np.asarray(guide, dtype=np.float32))
    weight = np.ascontiguousarray(np.asarray(weight, dtype=np.float32))
    bias = np.ascontiguousarray(np.asarray(bias, dtype=np.float32))
    in_maps = [
        {"x": x[b], "guide": guide[b], "weight": weight, "bias": bias}
        for b in range(B)
    ]
    res = run_bass_kernel_spmd(nc, in_maps, list(range(N_CORES))).results
    return np.stack([res[b]["out"] for b in range(B)], axis=0).astype(np.float32)
